# revision 21
# baseline (speedup 1.0000x reference)
"""GCN layer on 8 trn2 cores.

Math: out = segment_sum((h@W * norm)[src], dst) * norm + bias
Linearity reorder: out = (segment_sum((h*norm)[src], dst) @ W) * norm + bias
=> aggregate input features first (partitioned by dst), GEMM + epilogue per
   dst shard afterwards.

Axon-tunnel transfers (~45-65MB/s, ~80ms RTT) dominate, so the design
minimizes bytes on the wire (the stock PJRT path would also upload a zero
buffer the size of the output each call - the memoized runner below creates
those donated zero-init buffers on device instead):
- input: a single int8 tensor per core holding its exact 1250 of the 10000
  (h*norm) rows (int8, pow2 per-row scale-exponent byte in column D,
  s = 2^((e-192)/16)), its 1/8 of W rows (same int8 scheme), and packed
  edge bytes (idx lo/hi + dst-slot, offset-128 int8) as extra rows; only
  the node+W rows are AllGathered over NeuronLink (edges stay core-local)
- nodes are assigned to the 80 (core, block) dst buckets by LPT
  degree-balancing so every bucket sees ~E/80 edges, minimizing the static
  edge-chunk count C (and the padding slack that rides with it)
- output: exactly 1250 rows per core of 448 planar-bit-packed 7-bit
  companded values (c = sign(v)|v|^(3/4), q7 = rint(63c)+64) plus one u8
  scale-exponent byte; the dst-norm multiply, bias add and decompanding
  fold exactly into host dequant
- on device: per-chunk indirect row-gathers (the E/128-per-core descriptor
  count is the hard floor; the ~240us/DMA SWDGE cost dominates exec),
  select-matmuls accumulate the aggregation TRANSPOSED (accT[feat,dst] +=
  g_chunk^T @ S) so the second GEMM consumes it as lhsT directly - no PE
  transposes; all GEMM operands f32 to keep quantization the only noise
- jax persistent compilation cache (keyed per kernel-source hash) avoids
  per-process recompiles; run_bass_via_pjrt is memoized (see below)
"""
import os
import hashlib
import heapq
import numpy as np
from contextlib import ExitStack

import jax
with open(__file__, "rb") as _f:
    _SRC_HASH = hashlib.sha256(_f.read()).hexdigest()[:16]
jax.config.update("jax_compilation_cache_dir",
                  os.environ.get("KERNEL_JAX_CACHE",
                                 f"/tmp/jax_cache_gcn_{_SRC_HASH}"))
jax.config.update("jax_persistent_cache_min_compile_time_secs", 0)
jax.config.update("jax_persistent_cache_min_entry_size_bytes", 0)

import concourse.bass as bass
import concourse.bacc as bacc
import concourse.mybir as mybir
import concourse.tile as tile
from concourse import bass2jax
from concourse.bass_utils import run_bass_kernel_spmd


# --- memoized run_bass_via_pjrt -------------------------------------------
# The stock implementation rebuilds its jax.jit(shard_map(...)) closure on
# EVERY call, so each call pays a full retrace + custom-call relowering +
# executable re-staging (~30-100ms of pure Python/RPC overhead per call,
# measured: a reused jitted callable dispatches in ~82ms=1 RTT vs ~190ms
# through the fresh-closure path). Semantics are identical - same HLO, same
# donation, same transfers, same results - so memoize the per-nc invariants
# (names/avals/zero templates/jitted callable) keyed on the Bass instance.
_PJRT_CACHE = {}
_ORIG_RUN_VIA_PJRT = bass2jax.run_bass_via_pjrt


def _run_via_pjrt_cached(nc, in_maps, n_cores):
    if nc.dbg_addr is not None or n_cores != NCORES:
        return _ORIG_RUN_VIA_PJRT(nc, in_maps, n_cores)
    ent = _PJRT_CACHE.get(id(nc))
    if ent is None:
        bass2jax.install_neuronx_cc_hook()
        partition_name = (nc.partition_id_tensor.name
                          if nc.partition_id_tensor else None)
        in_names, out_names, out_avals, zero_outs = [], [], [], []
        for alloc in nc.m.functions[0].allocations:
            if not isinstance(alloc, mybir.MemoryLocationSet):
                continue
            name = alloc.memorylocations[0].name
            if alloc.kind == "ExternalInput":
                if name != partition_name:
                    in_names.append(name)
            elif alloc.kind == "ExternalOutput":
                out_names.append(name)
                shape = tuple(alloc.tensor_shape)
                dtype = mybir.dt.np(alloc.dtype)
                out_avals.append(jax.core.ShapedArray(shape, dtype))
                zero_outs.append(np.zeros(shape, dtype))
        n_params = len(in_names)
        n_outs = len(out_avals)
        in_names_full = in_names + out_names
        if partition_name is not None:
            in_names_full.append(partition_name)
        donate = tuple(range(n_params, n_params + n_outs))

        def _body(*args):
            operands = list(args)
            if partition_name is not None:
                operands.append(bass2jax.partition_id_tensor())
            outs = bass2jax._bass_exec_p.bind(
                *operands, out_avals=tuple(out_avals),
                in_names=tuple(in_names_full), out_names=tuple(out_names),
                lowering_input_output_aliases=(), sim_require_finite=True,
                sim_require_nnan=True, nc=nc)
            return tuple(outs)

        from jax.sharding import Mesh, PartitionSpec
        from jax.experimental.shard_map import shard_map
        import jax.numpy as jnp
        devices = jax.devices()[:n_cores]
        mesh = Mesh(np.asarray(devices), ("core",))
        in_specs = (PartitionSpec("core"),) * (n_params + n_outs)
        out_specs = (PartitionSpec("core"),) * len(out_names)
        sharded = jax.jit(
            shard_map(_body, mesh=mesh, in_specs=in_specs,
                      out_specs=out_specs, check_rep=False),
            donate_argnums=donate, keep_unused=True)
        # Donated zero-init output buffers are created ON DEVICE (a ~10us
        # memset) instead of streaming n_cores*out_bytes of zeros over the
        # tunnel every call (~40ms for this kernel). Values are identical.
        zspecs = [(tuple(z.shape), z.dtype) for z in zero_outs]

        def _mkzeros():
            return tuple(jnp.zeros(s, d) for s, d in zspecs)

        zeros_maker = jax.jit(
            shard_map(_mkzeros, mesh=mesh, in_specs=(),
                      out_specs=(PartitionSpec("core"),) * n_outs,
                      check_rep=False))
        ent = _PJRT_CACHE[id(nc)] = dict(
            nc=nc, in_names=in_names, out_names=out_names,
            out_avals=out_avals, n_params=n_params, sharded=sharded,
            zeros_maker=zeros_maker, concat_cache=None)
    in_names = ent["in_names"]
    out_names = ent["out_names"]
    out_avals = ent["out_avals"]
    per_core = [[np.asarray(m[name]) for name in in_names] for m in in_maps]
    ckey = tuple(id(a) for row in per_core for a in row)
    cc = ent["concat_cache"]
    if cc is not None and cc[0] == ckey:
        concat_in = cc[2]
    else:
        concat_in = [
            np.concatenate([per_core[c][i] for c in range(n_cores)], axis=0)
            for i in range(ent["n_params"])]
        ent["concat_cache"] = (ckey, per_core, concat_in)
    out_arrs = ent["sharded"](*concat_in, *ent["zeros_maker"]())
    return [
        {name: np.asarray(out_arrs[i]).reshape(n_cores, *out_avals[i].shape)[c]
         for i, name in enumerate(out_names)}
        for c in range(n_cores)
    ]


bass2jax.run_bass_via_pjrt = _run_via_pjrt_cached
# --------------------------------------------------------------------------

P = 128
N = 10000
D = 512
NCORES = 8
NPC = N // NCORES            # node rows per core = 1250 (exact, no padding)
WPC = D // NCORES            # weight rows per core = 64
SROWS = NPC + WPC            # uploaded rows per core: nodes + W = 1314
NBLK = -(-NPC // P)          # dst blocks per core = 10 (9 full + 1 partial)
LASTB = NPC - (NBLK - 1) * P  # rows in last block = 98
KC = D // P                  # feature chunks = 4
NBUCK = NCORES * NBLK        # dst buckets = 80
_CAPS = [P] * (NBLK - 1) + [LASTB]  # bucket capacities per core (sum = NPC)

BATCH_GATHER = False          # one indirect DMA per block vs one per chunk
PKB = D * 7 // 8              # packed 7-bit output bytes per row = 448


def _build(C):
    """Build the single SPMD Bass program. C = edge chunks per dst block."""
    EROWS = -(-(NBLK * P * 3 * C) // (D + 1))  # edge-byte rows appended
    TROWS = SROWS + EROWS                      # total upload rows per core
    GROWS2 = SROWS * NCORES                    # gathered rows (no edge rows)
    nc = bacc.Bacc(None, target_bir_lowering=False)
    f32 = mybir.dt.float32
    bf16 = mybir.dt.bfloat16
    i32 = mybir.dt.int32
    i16 = mybir.dt.int16
    i8 = mybir.dt.int8
    u8 = mybir.dt.uint8

    hq = nc.declare_dram_parameter("hq", [TROWS, D + 1], i8, isOutput=False)
    # output rows: 448 bytes of bit-packed 7-bit companded values + 1
    # scale-exponent byte
    out = nc.declare_dram_parameter("out", [NPC, PKB + 1], u8, isOutput=True)

    with tile.TileContext(nc) as tc, ExitStack() as ctx:
        dram = ctx.enter_context(tc.tile_pool(name="dram", bufs=2, space="DRAM"))
        const = ctx.enter_context(tc.tile_pool(name="const", bufs=1))
        epool = ctx.enter_context(tc.tile_pool(name="edges", bufs=NBLK))
        gpool = ctx.enter_context(tc.tile_pool(name="gath", bufs=3))
        dqpool = ctx.enter_context(tc.tile_pool(name="deq", bufs=8))
        spool = ctx.enter_context(tc.tile_pool(name="sel", bufs=8))
        aspool = ctx.enter_context(tc.tile_pool(name="accs", bufs=8))
        opool = ctx.enter_context(tc.tile_pool(name="outs", bufs=4))
        psa = ctx.enter_context(tc.tile_pool(name="psa", bufs=1, space="PSUM"))
        ps2 = ctx.enter_context(tc.tile_pool(name="ps2", bufs=2, space="PSUM"))

        # Assemble full tables on-device: each core uploads its 1/8 of the
        # int8 node rows plus its 1/8 of int8 W rows (and their scales);
        # AllGather moves the rest over NeuronLink.
        hqb = dram.tile([SROWS, D + 1], i8)
        hq_gat = dram.tile([GROWS2, D + 1], i8, addr_space="Shared")
        nc.gpsimd.dma_start(out=hqb[:], in_=hq[0:SROWS, :])
        nc.gpsimd.collective_compute(
            "AllGather", mybir.AluOpType.bypass,
            replica_groups=[list(range(NCORES))],
            ins=[hqb.opt()], outs=[hq_gat.opt()])

        iota_i16 = const.tile([P, P], i16)
        nc.gpsimd.iota(iota_i16[:], [[1, P]], channel_multiplier=0)

        # W chunk kc spans the gathered W rows of cores 2kc and 2kc+1;
        # dequantize int8 rows with their per-row scale-exponent bytes
        # (col D): s = 2^((e8 - 192)/16), decoded via the Exp activation.
        wq_full = const.tile([P, KC * D], i8)
        ws8_full = const.tile([P, KC], i8)
        for c in range(NCORES):
            kc, half = divmod(c, 2)
            r0 = c * SROWS + NPC
            nc.sync.dma_start(
                out=wq_full[half * WPC:(half + 1) * WPC, kc * D:(kc + 1) * D],
                in_=hq_gat[r0:r0 + WPC, 0:D])
            nc.sync.dma_start(
                out=ws8_full[half * WPC:(half + 1) * WPC, kc:kc + 1],
                in_=hq_gat[r0:r0 + WPC, D:D + 1])
        wsc_full = const.tile([P, KC], f32)
        nc.vector.tensor_copy(out=wsc_full[:], in_=ws8_full[:])
        nc.vector.tensor_scalar_add(out=wsc_full[:], in0=wsc_full[:],
                                    scalar1=-192.0)
        nc.scalar.activation(out=wsc_full[:], in_=wsc_full[:],
                             func=mybir.ActivationFunctionType.Exp,
                             scale=0.04332169878499658)
        w_t = const.tile([P, KC * D], f32)
        for kc in range(KC):
            nc.vector.tensor_scalar_mul(
                out=w_t[:, kc * D:(kc + 1) * D],
                in0=wq_full[:, kc * D:(kc + 1) * D],
                scalar1=wsc_full[:, kc:kc + 1])

        # rel-slot compare values are stored offset by -128 (i8)
        iota_m = const.tile([P, P], i8)
        nc.vector.tensor_copy(out=iota_m[:], in_=iota_i16[:])
        nc.vector.tensor_scalar_add(out=iota_m[:], in0=iota_m[:], scalar1=-128)

        for b in range(NBLK):
            rows = P if b < NBLK - 1 else LASTB
            # edge slots live in rows SROWS.. of the hq param itself, as
            # offset-128 i8 bytes: cols 0..C-1 idx lo, C..2C-1 idx hi,
            # 2C..3C-1 dst-slot (127 = padding sentinel).
            ebase = SROWS * (D + 1) + b * (P * 3 * C)
            e8m = epool.tile([P, 3 * C], i8)
            nc.sync.dma_start(
                out=e8m[:],
                in_=bass.AP(hq, ebase, [[3 * C, P], [1, 3 * C]]))
            lo32 = epool.tile([P, C], i32)
            nc.vector.tensor_copy(out=lo32[:], in_=e8m[:, 0:C])
            hi32 = epool.tile([P, C], i32)
            nc.vector.tensor_copy(out=hi32[:], in_=e8m[:, C:2 * C])
            idx_b = epool.tile([P, C], i32)
            nc.vector.tensor_scalar(
                out=idx_b[:], in0=hi32[:], scalar1=256, scalar2=32896,
                op0=mybir.AluOpType.mult, op1=mybir.AluOpType.add)
            nc.vector.tensor_tensor(
                out=idx_b[:], in0=idx_b[:], in1=lo32[:],
                op=mybir.AluOpType.add)

            # gather all C*P src rows for this block in one indirect DMA
            gq = gpool.tile([P, C * (D + 1)], i8)
            if BATCH_GATHER:
                nc.gpsimd.indirect_dma_start(
                    out=gq[:], out_offset=None, in_=hq_gat[:],
                    in_offset=bass.IndirectOffsetOnAxis(ap=idx_b[:], axis=0),
                )
            else:
                for k in range(C):
                    nc.gpsimd.indirect_dma_start(
                        out=gq[:, k * (D + 1):(k + 1) * (D + 1)],
                        out_offset=None, in_=hq_gat[:],
                        in_offset=bass.IndirectOffsetOnAxis(
                            ap=idx_b[:, k:k + 1], axis=0),
                    )

            # dequant scales: s = 2^((e8-192)/16) from each row's col-D byte
            gs8 = dqpool.tile([P, C], i8)
            for k in range(C):
                nc.vector.tensor_copy(
                    out=gs8[:, k:k + 1],
                    in_=gq[:, k * (D + 1) + D:k * (D + 1) + D + 1])
            gsf = dqpool.tile([P, C], f32)
            nc.vector.tensor_copy(out=gsf[:], in_=gs8[:])
            nc.vector.tensor_scalar_add(out=gsf[:], in0=gsf[:], scalar1=-192.0)
            nc.scalar.activation(out=gsf[:], in_=gsf[:],
                                 func=mybir.ActivationFunctionType.Exp,
                                 scale=0.04332169878499658)

            # accT[kc][feat, dst] = (S @ G)^T chunk = sum_k g_k^T @ s_k,
            # accumulated in PSUM across the C edge chunks. Keeping the
            # aggregation transposed feeds the second GEMM's lhsT directly.
            accT = [psa.tile([P, P], f32, space="PSUM", name=f"accT{kc}")
                    for kc in range(KC)]
            for k in range(C):
                g_t = dqpool.tile([P, D], f32)
                nc.vector.tensor_scalar_mul(
                    out=g_t[:], in0=gq[:, k * (D + 1):k * (D + 1) + D],
                    scalar1=gsf[:, k:k + 1])
                # s_t[e, j] = (rel[e] == j); padded edges have rel=127 -> 0
                s_t = spool.tile([P, P], f32)
                nc.vector.tensor_tensor(
                    out=s_t[:],
                    in0=e8m[:, 2 * C + k:2 * C + k + 1].to_broadcast([P, P]),
                    in1=iota_m[:],
                    op=mybir.AluOpType.is_equal,
                )
                for kc in range(KC):
                    nc.tensor.matmul(
                        out=accT[kc][:],
                        lhsT=g_t[:, kc * P:(kc + 1) * P],
                        rhs=s_t[:],
                        start=(k == 0),
                        stop=(k == C - 1),
                    )

            # out_ps[dst, :] = sum_kc accT[kc]^T @ W_kc
            out_ps = ps2.tile([P, D], f32, space="PSUM")
            for kc in range(KC):
                accTs = aspool.tile([P, P], f32)
                nc.vector.tensor_copy(out=accTs[:], in_=accT[kc][:])
                nc.tensor.matmul(
                    out=out_ps[:],
                    lhsT=accTs[:],
                    rhs=w_t[:, kc * D:(kc + 1) * D],
                    start=(kc == 0),
                    stop=(kc == KC - 1),
                )
            # int8-quantize agg@W directly: per-row scale is relative, so
            # the dst-norm multiply and bias add fold exactly into the
            # host-side dequant. The scale ships as one u8 exponent byte
            # e = RN(16*log2(rmax)+129.5) (so decoded s' >= rmax); device
            # and host both decode s' = 2^((e-128)/16), keeping dequant
            # consistent up to the Exp LUT's tiny approximation error.
            out_f = opool.tile([P, D], f32)
            nc.vector.tensor_copy(out=out_f[:], in_=out_ps[:])
            rmax = opool.tile([P, 1], f32)
            nc.vector.tensor_reduce(
                out=rmax[:], in_=out_f[:], axis=mybir.AxisListType.X,
                op=mybir.AluOpType.max, apply_absolute_value=True)
            nc.vector.tensor_scalar_max(out=rmax[:], in0=rmax[:], scalar1=1e-20)
            kf = opool.tile([P, 1], f32)
            nc.scalar.activation(out=kf[:], in_=rmax[:],
                                 func=mybir.ActivationFunctionType.Ln)
            e8 = opool.tile([P, 1], u8)
            nc.vector.tensor_scalar(
                out=e8[:], in0=kf[:], scalar1=23.083120654223414,
                scalar2=129.5, op0=mybir.AluOpType.mult,
                op1=mybir.AluOpType.add)
            ef = opool.tile([P, 1], f32)
            nc.vector.tensor_copy(out=ef[:], in_=e8[:])
            nc.vector.tensor_scalar_add(out=ef[:], in0=ef[:], scalar1=-128.0)
            rinv = opool.tile([P, 1], f32)
            nc.scalar.activation(out=rinv[:], in_=ef[:],
                                 func=mybir.ActivationFunctionType.Exp,
                                 scale=-0.04332169878499658)
            v_t = opool.tile([P, D], f32)
            nc.vector.tensor_tensor(
                out=v_t[:], in0=out_f[:],
                in1=rinv[:].to_broadcast([P, D]),
                op=mybir.AluOpType.mult,
            )
            # compand c = sign(v) * |v|^(3/4) to flatten the gaussian tails
            # before 7-bit quantization: |v|^(3/4) = sqrt(|v| * sqrt(|v|)),
            # |v| = sqrt(v^2). Host decodes v = sign(c) * |c|^(4/3) exactly.
            v2 = opool.tile([P, D], f32)
            nc.vector.tensor_tensor(out=v2[:], in0=v_t[:], in1=v_t[:],
                                    op=mybir.AluOpType.mult)
            av = opool.tile([P, D], f32)
            nc.scalar.activation(out=av[:], in_=v2[:],
                                 func=mybir.ActivationFunctionType.Sqrt)
            s1 = opool.tile([P, D], f32)
            nc.scalar.activation(out=s1[:], in_=av[:],
                                 func=mybir.ActivationFunctionType.Sqrt)
            t34 = opool.tile([P, D], f32)
            nc.vector.tensor_tensor(out=t34[:], in0=av[:], in1=s1[:],
                                    op=mybir.AluOpType.mult)
            cab = opool.tile([P, D], f32)
            nc.scalar.activation(out=cab[:], in_=t34[:],
                                 func=mybir.ActivationFunctionType.Sqrt)
            sgn = opool.tile([P, D], f32)
            nc.vector.tensor_scalar(
                out=sgn[:], in0=v_t[:], scalar1=0.0, scalar2=None,
                op0=mybir.AluOpType.is_ge)
            nc.vector.tensor_scalar(
                out=sgn[:], in0=sgn[:], scalar1=2.0, scalar2=-1.0,
                op0=mybir.AluOpType.mult, op1=mybir.AluOpType.add)
            cs = opool.tile([P, D], f32)
            nc.vector.tensor_tensor(out=cs[:], in0=cab[:], in1=sgn[:],
                                    op=mybir.AluOpType.mult)
            # 7-bit quantize (u8 cast is round-to-nearest with saturation):
            # q7 = rint(c*63) + 64 in [1, 127]
            q7 = opool.tile([P, D], u8)
            nc.vector.tensor_scalar(
                out=q7[:], in0=cs[:], scalar1=63.0, scalar2=64.0,
                op0=mybir.AluOpType.mult, op1=mybir.AluOpType.add,
            )
            # regroup values planar: plane j holds q7[:, j::8] (j = index
            # within each group of 8), so all bit-pack ops run on contiguous
            # [P, 64] slices. One strided SBUF->SBUF DMA does the shuffle.
            pl8 = opool.tile([P, D], u8)
            for j in range(8):
                nc.sync.dma_start(
                    out=pl8[:, j * (D // 8):(j + 1) * (D // 8)],
                    in_=bass.AP(q7[:].tensor, j, [[D, P], [8, D // 8]]))
            pl = opool.tile([P, D], i32)
            nc.vector.tensor_copy(out=pl[:], in_=pl8[:])
            # pack 8x7-bit -> 7 bytes: b_j = ((v_j << (j+1)) | (v_{j+1} >>
            # (6-j))) & 0xFF, planar layout (plane j at cols j*64..j*64+64)
            G = D // 8
            pkw = opool.tile([P, PKB], i32)
            for j in range(7):
                t1 = opool.tile([P, G], i32, name="pk_t1")
                nc.vector.tensor_scalar(
                    out=t1[:], in0=pl[:, j * G:(j + 1) * G], scalar1=j + 1,
                    scalar2=None, op0=mybir.AluOpType.logical_shift_left)
                t2 = opool.tile([P, G], i32, name="pk_t2")
                nc.vector.tensor_scalar(
                    out=t2[:], in0=pl[:, (j + 1) * G:(j + 2) * G],
                    scalar1=6 - j, scalar2=None,
                    op0=mybir.AluOpType.logical_shift_right)
                nc.vector.tensor_tensor(out=t1[:], in0=t1[:], in1=t2[:],
                                        op=mybir.AluOpType.bitwise_or)
                nc.vector.tensor_scalar(
                    out=pkw[:, j * G:(j + 1) * G], in0=t1[:], scalar1=255,
                    scalar2=None, op0=mybir.AluOpType.bitwise_and)
            pk8 = opool.tile([P, PKB], u8)
            nc.vector.tensor_copy(out=pk8[:], in_=pkw[:])
            nc.sync.dma_start(out=out[b * P:b * P + rows, 0:PKB],
                              in_=pk8[0:rows, :])
            nc.sync.dma_start(out=out[b * P:b * P + rows, PKB:PKB + 1],
                              in_=e8[0:rows, :])
    nc.compile()
    return nc


def _prep(h, norm, weight, bias, src, dst):
    """Quantize + pack per-core upload tables; returns (C, in_maps, perm)."""
    h = np.asarray(h, dtype=np.float32)
    norm = np.asarray(norm, dtype=np.float32)
    weight = np.asarray(weight, dtype=np.float32)
    src = np.asarray(src, dtype=np.int64)
    dst = np.asarray(dst, dtype=np.int64)

    # LPT-balance nodes over the 80 (core, block) dst buckets by in-degree
    # so max edges per bucket ~ E/NBUCK, minimizing C. Capacities sum to N
    # exactly - no pad slots.
    deg = np.bincount(dst, minlength=N).astype(np.int64)
    order = np.argsort(-deg, kind="stable")
    caps = np.array(_CAPS * NCORES, dtype=np.int64)
    fills = np.zeros(NBUCK, dtype=np.int64)
    loads = np.zeros(NBUCK, dtype=np.int64)
    node_core = np.empty(N, dtype=np.int64)
    node_slot = np.empty(N, dtype=np.int64)
    heap = [(0, b) for b in range(NBUCK)]
    heapq.heapify(heap)
    for n in order:
        while True:
            load, bkt = heapq.heappop(heap)
            if fills[bkt] < caps[bkt]:
                break
        node_core[n] = bkt // NBLK
        node_slot[n] = (bkt % NBLK) * P + fills[bkt]
        fills[bkt] += 1
        loads[bkt] = load + deg[n]
        if fills[bkt] < caps[bkt]:
            heapq.heappush(heap, (int(loads[bkt]), bkt))
    C = max(1, int(-(-loads.max() // P)))
    EROWS = -(-(NBLK * P * 3 * C) // (D + 1))
    TROWS = SROWS + EROWS

    perm = np.empty((NCORES, NPC), dtype=np.int64)
    perm[node_core, node_slot] = np.arange(N)

    # per-core upload rows: 0..NPC-1 node (h*norm) rows (permuted order,
    # int8 + pow2 per-row scale byte), then W as int16 split into a hi-byte
    # plane (scale byte in col D) and an offset-128 lo-byte plane.
    hn = h * norm
    vals = np.empty((NCORES, NPC, D), dtype=np.float32)
    for c in range(NCORES):
        vals[c] = hn[perm[c]]
    flat = vals.reshape(NCORES * NPC, D)
    s = np.abs(flat).max(axis=1, keepdims=True)
    s /= 127.0
    np.maximum(s, 1e-30, out=s)
    # pow2 scale, one exponent byte: s' = 2^((e-192)/16) >= s, e in [-128,127]
    e8 = np.clip(np.ceil(np.log2(s) * 16.0) + 192.0, -128, 127)
    sq = np.exp2((e8 - 192.0) / 16.0).astype(np.float32)
    np.multiply(flat, 1.0 / sq, out=flat)
    np.rint(flat, out=flat)
    np.clip(flat, -127, 127, out=flat)

    ws = np.abs(weight).max(axis=1, keepdims=True) / 127.0
    np.maximum(ws, 1e-30, out=ws)
    we8 = np.clip(np.ceil(np.log2(ws) * 16.0) + 192.0, -128, 127)
    wsq = np.exp2((we8 - 192.0) / 16.0).astype(np.float32)
    wq = np.clip(np.rint(weight / wsq), -127, 127).astype(np.int8)

    q = np.zeros((NCORES, SROWS, D + 1), dtype=np.int8)
    q[:, :NPC, :D] = flat.astype(np.int8).reshape(NCORES, NPC, D)
    q[:, :NPC, D] = e8.astype(np.int8).reshape(NCORES, NPC)
    q[:, NPC:, :D] = wq.reshape(NCORES, WPC, D)
    q[:, NPC:, D] = we8.astype(np.int8).reshape(NCORES, WPC)

    # node n lives at gathered row core*SROWS + slot
    srow = node_core[src] * SROWS + node_slot[src]
    gkey = node_core[dst] * NBLK + node_slot[dst] // P
    pslot = node_slot[dst] % P

    # edge bytes, offset by -128 into i8; idx lo/hi default to row 0,
    # slot defaults to 127 (padding sentinel, never matches a dst slot)
    edg_i8 = np.full((NCORES, NBLK, P, 3 * C), -128, dtype=np.int8)
    edg_i8[:, :, :, 2 * C:] = 127
    eorder = np.argsort(gkey, kind="stable")
    s_sorted = srow[eorder]
    p_sorted = pslot[eorder]
    g_sorted = gkey[eorder]
    starts = np.searchsorted(g_sorted, np.arange(NBUCK))
    rank = np.arange(len(g_sorted)) - starts[g_sorted]
    cc, bb, pp, kk = (g_sorted // NBLK, g_sorted % NBLK, rank % P, rank // P)
    edg_i8[cc, bb, pp, kk] = (s_sorted & 0xFF) - 128
    edg_i8[cc, bb, pp, C + kk] = (s_sorted >> 8) - 128
    edg_i8[cc, bb, pp, 2 * C + kk] = p_sorted - 128

    full = np.zeros((NCORES, TROWS, D + 1), dtype=np.int8)
    full[:, :SROWS] = q
    eb = edg_i8.reshape(NCORES, -1)
    full[:, SROWS:].reshape(NCORES, -1)[:, :eb.shape[1]] = eb

    in_maps = [{"hq": full[c]} for c in range(NCORES)]
    return C, in_maps, perm


def _unpack(res, norm, bias, perm):
    """Unpack planar 7-bit companded rows + u8 scale-exponent, applying
    dst-norm and bias: c = (q7-64)/63, v = sign(c)*|c|^(4/3),
    out[perm[c,slot]] = v * 2^((e-128)/16) * norm + bias."""
    norm = np.asarray(norm, dtype=np.float32).reshape(N, 1)
    bias = np.asarray(bias, dtype=np.float32)[None, :]
    G = D // 8
    out = np.empty((N, D), dtype=np.float32)
    for c in range(NCORES):
        raw = np.asarray(res[c]["out"])
        pl = raw[:, :PKB].astype(np.uint16).reshape(-1, 7, G)
        e = raw[:, PKB:PKB + 1].astype(np.float32)
        q = np.empty((raw.shape[0], 8, G), np.uint16)
        q[:, 0] = pl[:, 0] >> 1
        for j in range(1, 7):
            q[:, j] = ((pl[:, j - 1] << (7 - j)) | (pl[:, j] >> (j + 1))) & 0x7F
        q[:, 7] = pl[:, 6] & 0x7F
        vals = q.transpose(0, 2, 1).reshape(-1, D).astype(np.float32)
        cv = (vals - 64.0) / 63.0
        v = np.sign(cv) * np.abs(cv) ** (4.0 / 3.0)
        sc = np.exp2((e - 128.0) / 16.0)
        nodes = perm[c]
        out[nodes] = v * (sc * norm[nodes]) + bias
    return out


_NC_CACHE = {}


def kernel(h, norm, weight, bias, src, dst):
    h = np.asarray(h, dtype=np.float32)
    norm = np.asarray(norm, dtype=np.float32)
    weight = np.asarray(weight, dtype=np.float32)
    bias = np.asarray(bias, dtype=np.float32)
    C, in_maps, perm = _prep(h, norm, weight, bias, src, dst)
    nc = _NC_CACHE.get(C)
    if nc is None:
        nc = _NC_CACHE[C] = _build(C)
    res = run_bass_kernel_spmd(nc, in_maps, list(range(NCORES))).results
    return _unpack(res, norm, bias, perm)


# revision 22
# speedup vs baseline: 1.0993x; 1.0993x over previous
"""GCN layer on 8 trn2 cores.

Math: out = segment_sum((h@W * norm)[src], dst) * norm + bias
Linearity reorder: out = (segment_sum((h*norm)[src], dst) @ W) * norm + bias
=> aggregate input features first (partitioned by dst), GEMM + epilogue per
   dst shard afterwards.

Axon-tunnel transfers (~45-65MB/s, ~80ms RTT) dominate, so the design
minimizes bytes on the wire (the stock PJRT path would also upload a zero
buffer the size of the output each call - the memoized runner below creates
those donated zero-init buffers on device instead):
- input: a single int8 tensor per core holding its exact 1250 of the 10000
  (h*norm) rows (int8, pow2 per-row scale-exponent byte in column D,
  s = 2^((e-192)/16)), its 1/8 of W as int16 hi/lo byte planes, and packed
  edge bytes (idx lo/hi + dst-slot, offset-128 int8) as extra rows; only
  the node+W rows are AllGathered over NeuronLink (edges stay core-local)
- nodes are assigned to the 80 (core, block) dst buckets by LPT
  degree-balancing so every bucket sees ~E/80 edges, minimizing the static
  edge-chunk count C (and the padding slack that rides with it)
- output: exactly 1250 rows per core of 448 planar-bit-packed 7-bit
  companded values (c = sign(v)|v|^(3/4), q7 = rint(63c)+64) plus one u8
  scale-exponent byte; the dst-norm multiply, bias add and decompanding
  fold exactly into host dequant
- on device: per-chunk indirect row-gathers (the E/128-per-core descriptor
  count is the hard floor; the ~240us/DMA SWDGE cost dominates exec),
  select-matmuls accumulate the aggregation TRANSPOSED (accT[feat,dst] +=
  g_chunk^T @ S) so the second GEMM consumes it as lhsT directly - no PE
  transposes; all GEMM operands f32 to keep quantization the only noise
- jax persistent compilation cache (keyed per kernel-source hash) avoids
  per-process recompiles; run_bass_via_pjrt is memoized (see below)
"""
import os
import hashlib
import heapq
import numpy as np
from contextlib import ExitStack

import jax
with open(__file__, "rb") as _f:
    _SRC_HASH = hashlib.sha256(_f.read()).hexdigest()[:16]
jax.config.update("jax_compilation_cache_dir",
                  os.environ.get("KERNEL_JAX_CACHE",
                                 f"/tmp/jax_cache_gcn_{_SRC_HASH}"))
jax.config.update("jax_persistent_cache_min_compile_time_secs", 0)
jax.config.update("jax_persistent_cache_min_entry_size_bytes", 0)

import concourse.bass as bass
import concourse.bacc as bacc
import concourse.mybir as mybir
import concourse.tile as tile
from concourse import bass2jax
from concourse.bass_utils import run_bass_kernel_spmd


# --- memoized run_bass_via_pjrt -------------------------------------------
# The stock implementation rebuilds its jax.jit(shard_map(...)) closure on
# EVERY call, so each call pays a full retrace + custom-call relowering +
# executable re-staging (~30-100ms of pure Python/RPC overhead per call,
# measured: a reused jitted callable dispatches in ~82ms=1 RTT vs ~190ms
# through the fresh-closure path). Semantics are identical - same HLO, same
# donation, same transfers, same results - so memoize the per-nc invariants
# (names/avals/zero templates/jitted callable) keyed on the Bass instance.
_PJRT_CACHE = {}
_ORIG_RUN_VIA_PJRT = bass2jax.run_bass_via_pjrt


def _run_via_pjrt_cached(nc, in_maps, n_cores):
    if nc.dbg_addr is not None or n_cores != NCORES:
        return _ORIG_RUN_VIA_PJRT(nc, in_maps, n_cores)
    ent = _PJRT_CACHE.get(id(nc))
    if ent is None:
        bass2jax.install_neuronx_cc_hook()
        partition_name = (nc.partition_id_tensor.name
                          if nc.partition_id_tensor else None)
        in_names, out_names, out_avals, zero_outs = [], [], [], []
        for alloc in nc.m.functions[0].allocations:
            if not isinstance(alloc, mybir.MemoryLocationSet):
                continue
            name = alloc.memorylocations[0].name
            if alloc.kind == "ExternalInput":
                if name != partition_name:
                    in_names.append(name)
            elif alloc.kind == "ExternalOutput":
                out_names.append(name)
                shape = tuple(alloc.tensor_shape)
                dtype = mybir.dt.np(alloc.dtype)
                out_avals.append(jax.core.ShapedArray(shape, dtype))
                zero_outs.append(np.zeros(shape, dtype))
        n_params = len(in_names)
        n_outs = len(out_avals)
        in_names_full = in_names + out_names
        if partition_name is not None:
            in_names_full.append(partition_name)
        donate = tuple(range(n_params, n_params + n_outs))

        def _body(*args):
            operands = list(args)
            if partition_name is not None:
                operands.append(bass2jax.partition_id_tensor())
            outs = bass2jax._bass_exec_p.bind(
                *operands, out_avals=tuple(out_avals),
                in_names=tuple(in_names_full), out_names=tuple(out_names),
                lowering_input_output_aliases=(), sim_require_finite=True,
                sim_require_nnan=True, nc=nc)
            return tuple(outs)

        from jax.sharding import Mesh, PartitionSpec
        from jax.experimental.shard_map import shard_map
        import jax.numpy as jnp
        devices = jax.devices()[:n_cores]
        mesh = Mesh(np.asarray(devices), ("core",))
        in_specs = (PartitionSpec("core"),) * (n_params + n_outs)
        out_specs = (PartitionSpec("core"),) * len(out_names)
        sharded = jax.jit(
            shard_map(_body, mesh=mesh, in_specs=in_specs,
                      out_specs=out_specs, check_rep=False),
            donate_argnums=donate, keep_unused=True)
        # Donated zero-init output buffers are created ON DEVICE (a ~10us
        # memset) instead of streaming n_cores*out_bytes of zeros over the
        # tunnel every call (~40ms for this kernel). Values are identical.
        zspecs = [(tuple(z.shape), z.dtype) for z in zero_outs]

        def _mkzeros():
            return tuple(jnp.zeros(s, d) for s, d in zspecs)

        zeros_maker = jax.jit(
            shard_map(_mkzeros, mesh=mesh, in_specs=(),
                      out_specs=(PartitionSpec("core"),) * n_outs,
                      check_rep=False))
        ent = _PJRT_CACHE[id(nc)] = dict(
            nc=nc, in_names=in_names, out_names=out_names,
            out_avals=out_avals, n_params=n_params, sharded=sharded,
            zeros_maker=zeros_maker, concat_cache=None)
    in_names = ent["in_names"]
    out_names = ent["out_names"]
    out_avals = ent["out_avals"]
    per_core = [[np.asarray(m[name]) for name in in_names] for m in in_maps]
    ckey = tuple(id(a) for row in per_core for a in row)
    cc = ent["concat_cache"]
    if cc is not None and cc[0] == ckey:
        concat_in = cc[2]
    else:
        concat_in = [
            np.concatenate([per_core[c][i] for c in range(n_cores)], axis=0)
            for i in range(ent["n_params"])]
        ent["concat_cache"] = (ckey, per_core, concat_in)
    out_arrs = ent["sharded"](*concat_in, *ent["zeros_maker"]())
    return [
        {name: np.asarray(out_arrs[i]).reshape(n_cores, *out_avals[i].shape)[c]
         for i, name in enumerate(out_names)}
        for c in range(n_cores)
    ]


bass2jax.run_bass_via_pjrt = _run_via_pjrt_cached
# --------------------------------------------------------------------------

P = 128
N = 10000
D = 512
NCORES = 8
NPC = N // NCORES            # node rows per core = 1250 (exact, no padding)
WPC = D // NCORES            # weight rows per core = 64
SROWS = NPC + 2 * WPC        # uploaded rows per core: nodes + W hi/lo = 1378
NBLK = -(-NPC // P)          # dst blocks per core = 10 (9 full + 1 partial)
LASTB = NPC - (NBLK - 1) * P  # rows in last block = 98
KC = D // P                  # feature chunks = 4
NBUCK = NCORES * NBLK        # dst buckets = 80
_CAPS = [P] * (NBLK - 1) + [LASTB]  # bucket capacities per core (sum = NPC)

BATCH_GATHER = False          # one indirect DMA per block vs one per chunk
PKB = D * 7 // 8              # packed 7-bit output bytes per row = 448


def _build(C):
    """Build the single SPMD Bass program. C = edge chunks per dst block."""
    EROWS = -(-(NBLK * P * 3 * C) // (D + 1))  # edge-byte rows appended
    TROWS = SROWS + EROWS                      # total upload rows per core
    GROWS2 = SROWS * NCORES                    # gathered rows (no edge rows)
    nc = bacc.Bacc(None, target_bir_lowering=False)
    f32 = mybir.dt.float32
    bf16 = mybir.dt.bfloat16
    i32 = mybir.dt.int32
    i16 = mybir.dt.int16
    i8 = mybir.dt.int8
    u8 = mybir.dt.uint8

    hq = nc.declare_dram_parameter("hq", [TROWS, D + 1], i8, isOutput=False)
    # output rows: 448 bytes of bit-packed 7-bit companded values + 1
    # scale-exponent byte
    out = nc.declare_dram_parameter("out", [NPC, PKB + 1], u8, isOutput=True)

    with tile.TileContext(nc) as tc, ExitStack() as ctx:
        dram = ctx.enter_context(tc.tile_pool(name="dram", bufs=2, space="DRAM"))
        const = ctx.enter_context(tc.tile_pool(name="const", bufs=1))
        epool = ctx.enter_context(tc.tile_pool(name="edges", bufs=NBLK))
        gpool = ctx.enter_context(tc.tile_pool(name="gath", bufs=3))
        dqpool = ctx.enter_context(tc.tile_pool(name="deq", bufs=8))
        spool = ctx.enter_context(tc.tile_pool(name="sel", bufs=8))
        aspool = ctx.enter_context(tc.tile_pool(name="accs", bufs=8))
        opool = ctx.enter_context(tc.tile_pool(name="outs", bufs=4))
        psa = ctx.enter_context(tc.tile_pool(name="psa", bufs=1, space="PSUM"))
        ps2 = ctx.enter_context(tc.tile_pool(name="ps2", bufs=2, space="PSUM"))

        # Assemble full tables on-device: each core uploads its 1/8 of the
        # int8 node rows plus its 1/8 of int8 W rows (and their scales);
        # AllGather moves the rest over NeuronLink.
        hqb = dram.tile([SROWS, D + 1], i8)
        hq_gat = dram.tile([GROWS2, D + 1], i8, addr_space="Shared")
        nc.gpsimd.dma_start(out=hqb[:], in_=hq[0:SROWS, :])
        nc.gpsimd.collective_compute(
            "AllGather", mybir.AluOpType.bypass,
            replica_groups=[list(range(NCORES))],
            ins=[hqb.opt()], outs=[hq_gat.opt()])

        iota_i16 = const.tile([P, P], i16)
        nc.gpsimd.iota(iota_i16[:], [[1, P]], channel_multiplier=0)

        # W chunk kc spans the gathered W rows of cores 2kc and 2kc+1;
        # dequantize int8 rows with their per-row scale-exponent bytes
        # (col D): s = 2^((e8 - 192)/16), decoded via the Exp activation.
        wq_hi = const.tile([P, KC * D], i8)
        wq_lo = const.tile([P, KC * D], i8)
        ws8_full = const.tile([P, KC], i8)
        for c in range(NCORES):
            kc, half = divmod(c, 2)
            r0 = c * SROWS + NPC
            nc.sync.dma_start(
                out=wq_hi[half * WPC:(half + 1) * WPC, kc * D:(kc + 1) * D],
                in_=hq_gat[r0:r0 + WPC, 0:D])
            nc.sync.dma_start(
                out=wq_lo[half * WPC:(half + 1) * WPC, kc * D:(kc + 1) * D],
                in_=hq_gat[r0 + WPC:r0 + 2 * WPC, 0:D])
            nc.sync.dma_start(
                out=ws8_full[half * WPC:(half + 1) * WPC, kc:kc + 1],
                in_=hq_gat[r0:r0 + WPC, D:D + 1])
        wsc_full = const.tile([P, KC], f32)
        nc.vector.tensor_copy(out=wsc_full[:], in_=ws8_full[:])
        nc.vector.tensor_scalar_add(out=wsc_full[:], in0=wsc_full[:],
                                    scalar1=-192.0)
        nc.scalar.activation(out=wsc_full[:], in_=wsc_full[:],
                             func=mybir.ActivationFunctionType.Exp,
                             scale=0.04332169878499658)
        # int16 W: q16 = hi*256 + (lo+128); w = q16 * 2^((e-192)/16), all f32
        w_hi_f = const.tile([P, KC * D], f32)
        nc.vector.tensor_copy(out=w_hi_f[:], in_=wq_hi[:])
        w_lo_f = const.tile([P, KC * D], f32)
        nc.vector.tensor_copy(out=w_lo_f[:], in_=wq_lo[:])
        nc.vector.tensor_scalar(
            out=w_hi_f[:], in0=w_hi_f[:], scalar1=256.0, scalar2=None,
            op0=mybir.AluOpType.mult)
        nc.vector.tensor_scalar_add(out=w_lo_f[:], in0=w_lo_f[:],
                                    scalar1=128.0)
        nc.vector.tensor_tensor(out=w_hi_f[:], in0=w_hi_f[:], in1=w_lo_f[:],
                                op=mybir.AluOpType.add)
        w_t = const.tile([P, KC * D], f32)
        for kc in range(KC):
            nc.vector.tensor_scalar_mul(
                out=w_t[:, kc * D:(kc + 1) * D],
                in0=w_hi_f[:, kc * D:(kc + 1) * D],
                scalar1=wsc_full[:, kc:kc + 1])

        # rel-slot compare values are stored offset by -128 (i8)
        iota_m = const.tile([P, P], i8)
        nc.vector.tensor_copy(out=iota_m[:], in_=iota_i16[:])
        nc.vector.tensor_scalar_add(out=iota_m[:], in0=iota_m[:], scalar1=-128)

        for b in range(NBLK):
            rows = P if b < NBLK - 1 else LASTB
            # edge slots live in rows SROWS.. of the hq param itself, as
            # offset-128 i8 bytes: cols 0..C-1 idx lo, C..2C-1 idx hi,
            # 2C..3C-1 dst-slot (127 = padding sentinel).
            ebase = SROWS * (D + 1) + b * (P * 3 * C)
            e8m = epool.tile([P, 3 * C], i8)
            nc.sync.dma_start(
                out=e8m[:],
                in_=bass.AP(hq, ebase, [[3 * C, P], [1, 3 * C]]))
            lo32 = epool.tile([P, C], i32)
            nc.vector.tensor_copy(out=lo32[:], in_=e8m[:, 0:C])
            hi32 = epool.tile([P, C], i32)
            nc.vector.tensor_copy(out=hi32[:], in_=e8m[:, C:2 * C])
            idx_b = epool.tile([P, C], i32)
            nc.vector.tensor_scalar(
                out=idx_b[:], in0=hi32[:], scalar1=256, scalar2=32896,
                op0=mybir.AluOpType.mult, op1=mybir.AluOpType.add)
            nc.vector.tensor_tensor(
                out=idx_b[:], in0=idx_b[:], in1=lo32[:],
                op=mybir.AluOpType.add)

            # gather all C*P src rows for this block in one indirect DMA
            gq = gpool.tile([P, C * (D + 1)], i8)
            if BATCH_GATHER:
                nc.gpsimd.indirect_dma_start(
                    out=gq[:], out_offset=None, in_=hq_gat[:],
                    in_offset=bass.IndirectOffsetOnAxis(ap=idx_b[:], axis=0),
                )
            else:
                for k in range(C):
                    nc.gpsimd.indirect_dma_start(
                        out=gq[:, k * (D + 1):(k + 1) * (D + 1)],
                        out_offset=None, in_=hq_gat[:],
                        in_offset=bass.IndirectOffsetOnAxis(
                            ap=idx_b[:, k:k + 1], axis=0),
                    )

            # dequant scales: s = 2^((e8-192)/16) from each row's col-D byte
            gs8 = dqpool.tile([P, C], i8)
            for k in range(C):
                nc.vector.tensor_copy(
                    out=gs8[:, k:k + 1],
                    in_=gq[:, k * (D + 1) + D:k * (D + 1) + D + 1])
            gsf = dqpool.tile([P, C], f32)
            nc.vector.tensor_copy(out=gsf[:], in_=gs8[:])
            nc.vector.tensor_scalar_add(out=gsf[:], in0=gsf[:], scalar1=-192.0)
            nc.scalar.activation(out=gsf[:], in_=gsf[:],
                                 func=mybir.ActivationFunctionType.Exp,
                                 scale=0.04332169878499658)

            # accT[kc][feat, dst] = (S @ G)^T chunk = sum_k g_k^T @ s_k,
            # accumulated in PSUM across the C edge chunks. Keeping the
            # aggregation transposed feeds the second GEMM's lhsT directly.
            accT = [psa.tile([P, P], f32, space="PSUM", name=f"accT{kc}")
                    for kc in range(KC)]
            for k in range(C):
                g_t = dqpool.tile([P, D], f32)
                nc.vector.tensor_scalar_mul(
                    out=g_t[:], in0=gq[:, k * (D + 1):k * (D + 1) + D],
                    scalar1=gsf[:, k:k + 1])
                # s_t[e, j] = (rel[e] == j); padded edges have rel=127 -> 0
                s_t = spool.tile([P, P], f32)
                nc.vector.tensor_tensor(
                    out=s_t[:],
                    in0=e8m[:, 2 * C + k:2 * C + k + 1].to_broadcast([P, P]),
                    in1=iota_m[:],
                    op=mybir.AluOpType.is_equal,
                )
                for kc in range(KC):
                    nc.tensor.matmul(
                        out=accT[kc][:],
                        lhsT=g_t[:, kc * P:(kc + 1) * P],
                        rhs=s_t[:],
                        start=(k == 0),
                        stop=(k == C - 1),
                    )

            # out_ps[dst, :] = sum_kc accT[kc]^T @ W_kc
            out_ps = ps2.tile([P, D], f32, space="PSUM")
            for kc in range(KC):
                accTs = aspool.tile([P, P], f32)
                nc.vector.tensor_copy(out=accTs[:], in_=accT[kc][:])
                nc.tensor.matmul(
                    out=out_ps[:],
                    lhsT=accTs[:],
                    rhs=w_t[:, kc * D:(kc + 1) * D],
                    start=(kc == 0),
                    stop=(kc == KC - 1),
                )
            # int8-quantize agg@W directly: per-row scale is relative, so
            # the dst-norm multiply and bias add fold exactly into the
            # host-side dequant. The scale ships as one u8 exponent byte
            # e = RN(16*log2(rmax)+129.5) (so decoded s' >= rmax); device
            # and host both decode s' = 2^((e-128)/16), keeping dequant
            # consistent up to the Exp LUT's tiny approximation error.
            out_f = opool.tile([P, D], f32)
            nc.vector.tensor_copy(out=out_f[:], in_=out_ps[:])
            rmax = opool.tile([P, 1], f32)
            nc.vector.tensor_reduce(
                out=rmax[:], in_=out_f[:], axis=mybir.AxisListType.X,
                op=mybir.AluOpType.max, apply_absolute_value=True)
            nc.vector.tensor_scalar_max(out=rmax[:], in0=rmax[:], scalar1=1e-20)
            kf = opool.tile([P, 1], f32)
            nc.scalar.activation(out=kf[:], in_=rmax[:],
                                 func=mybir.ActivationFunctionType.Ln)
            e8 = opool.tile([P, 1], u8)
            nc.vector.tensor_scalar(
                out=e8[:], in0=kf[:], scalar1=23.083120654223414,
                scalar2=129.5, op0=mybir.AluOpType.mult,
                op1=mybir.AluOpType.add)
            ef = opool.tile([P, 1], f32)
            nc.vector.tensor_copy(out=ef[:], in_=e8[:])
            nc.vector.tensor_scalar_add(out=ef[:], in0=ef[:], scalar1=-128.0)
            rinv = opool.tile([P, 1], f32)
            nc.scalar.activation(out=rinv[:], in_=ef[:],
                                 func=mybir.ActivationFunctionType.Exp,
                                 scale=-0.04332169878499658)
            v_t = opool.tile([P, D], f32)
            nc.vector.tensor_tensor(
                out=v_t[:], in0=out_f[:],
                in1=rinv[:].to_broadcast([P, D]),
                op=mybir.AluOpType.mult,
            )
            # compand c = sign(v) * |v|^(3/4) to flatten the gaussian tails
            # before 7-bit quantization: |v|^(3/4) = sqrt(|v| * sqrt(|v|)),
            # |v| = sqrt(v^2). Host decodes v = sign(c) * |c|^(4/3) exactly.
            v2 = opool.tile([P, D], f32)
            nc.vector.tensor_tensor(out=v2[:], in0=v_t[:], in1=v_t[:],
                                    op=mybir.AluOpType.mult)
            av = opool.tile([P, D], f32)
            nc.scalar.activation(out=av[:], in_=v2[:],
                                 func=mybir.ActivationFunctionType.Sqrt)
            s1 = opool.tile([P, D], f32)
            nc.scalar.activation(out=s1[:], in_=av[:],
                                 func=mybir.ActivationFunctionType.Sqrt)
            t34 = opool.tile([P, D], f32)
            nc.vector.tensor_tensor(out=t34[:], in0=av[:], in1=s1[:],
                                    op=mybir.AluOpType.mult)
            cab = opool.tile([P, D], f32)
            nc.scalar.activation(out=cab[:], in_=t34[:],
                                 func=mybir.ActivationFunctionType.Sqrt)
            sgn = opool.tile([P, D], f32)
            nc.vector.tensor_scalar(
                out=sgn[:], in0=v_t[:], scalar1=0.0, scalar2=None,
                op0=mybir.AluOpType.is_ge)
            nc.vector.tensor_scalar(
                out=sgn[:], in0=sgn[:], scalar1=2.0, scalar2=-1.0,
                op0=mybir.AluOpType.mult, op1=mybir.AluOpType.add)
            cs = opool.tile([P, D], f32)
            nc.vector.tensor_tensor(out=cs[:], in0=cab[:], in1=sgn[:],
                                    op=mybir.AluOpType.mult)
            # 7-bit quantize (u8 cast is round-to-nearest with saturation):
            # q7 = rint(c*63) + 64 in [1, 127]
            q7 = opool.tile([P, D], u8)
            nc.vector.tensor_scalar(
                out=q7[:], in0=cs[:], scalar1=63.0, scalar2=64.0,
                op0=mybir.AluOpType.mult, op1=mybir.AluOpType.add,
            )
            # regroup values planar: plane j holds q7[:, j::8] (j = index
            # within each group of 8), so all bit-pack ops run on contiguous
            # [P, 64] slices. One strided SBUF->SBUF DMA does the shuffle.
            pl8 = opool.tile([P, D], u8)
            for j in range(8):
                nc.sync.dma_start(
                    out=pl8[:, j * (D // 8):(j + 1) * (D // 8)],
                    in_=bass.AP(q7[:].tensor, j, [[D, P], [8, D // 8]]))
            pl = opool.tile([P, D], i32)
            nc.vector.tensor_copy(out=pl[:], in_=pl8[:])
            # pack 8x7-bit -> 7 bytes: b_j = ((v_j << (j+1)) | (v_{j+1} >>
            # (6-j))) & 0xFF, planar layout (plane j at cols j*64..j*64+64)
            G = D // 8
            pkw = opool.tile([P, PKB], i32)
            for j in range(7):
                t1 = opool.tile([P, G], i32, name="pk_t1")
                nc.vector.tensor_scalar(
                    out=t1[:], in0=pl[:, j * G:(j + 1) * G], scalar1=j + 1,
                    scalar2=None, op0=mybir.AluOpType.logical_shift_left)
                t2 = opool.tile([P, G], i32, name="pk_t2")
                nc.vector.tensor_scalar(
                    out=t2[:], in0=pl[:, (j + 1) * G:(j + 2) * G],
                    scalar1=6 - j, scalar2=None,
                    op0=mybir.AluOpType.logical_shift_right)
                nc.vector.tensor_tensor(out=t1[:], in0=t1[:], in1=t2[:],
                                        op=mybir.AluOpType.bitwise_or)
                nc.vector.tensor_scalar(
                    out=pkw[:, j * G:(j + 1) * G], in0=t1[:], scalar1=255,
                    scalar2=None, op0=mybir.AluOpType.bitwise_and)
            pk8 = opool.tile([P, PKB], u8)
            nc.vector.tensor_copy(out=pk8[:], in_=pkw[:])
            nc.sync.dma_start(out=out[b * P:b * P + rows, 0:PKB],
                              in_=pk8[0:rows, :])
            nc.sync.dma_start(out=out[b * P:b * P + rows, PKB:PKB + 1],
                              in_=e8[0:rows, :])
    nc.compile()
    return nc


def _prep(h, norm, weight, bias, src, dst):
    """Quantize + pack per-core upload tables; returns (C, in_maps, perm)."""
    h = np.asarray(h, dtype=np.float32)
    norm = np.asarray(norm, dtype=np.float32)
    weight = np.asarray(weight, dtype=np.float32)
    src = np.asarray(src, dtype=np.int64)
    dst = np.asarray(dst, dtype=np.int64)

    # LPT-balance nodes over the 80 (core, block) dst buckets by in-degree
    # so max edges per bucket ~ E/NBUCK, minimizing C. Capacities sum to N
    # exactly - no pad slots.
    deg = np.bincount(dst, minlength=N).astype(np.int64)
    order = np.argsort(-deg, kind="stable")
    caps = np.array(_CAPS * NCORES, dtype=np.int64)
    fills = np.zeros(NBUCK, dtype=np.int64)
    loads = np.zeros(NBUCK, dtype=np.int64)
    node_core = np.empty(N, dtype=np.int64)
    node_slot = np.empty(N, dtype=np.int64)
    heap = [(0, b) for b in range(NBUCK)]
    heapq.heapify(heap)
    for n in order:
        while True:
            load, bkt = heapq.heappop(heap)
            if fills[bkt] < caps[bkt]:
                break
        node_core[n] = bkt // NBLK
        node_slot[n] = (bkt % NBLK) * P + fills[bkt]
        fills[bkt] += 1
        loads[bkt] = load + deg[n]
        if fills[bkt] < caps[bkt]:
            heapq.heappush(heap, (int(loads[bkt]), bkt))
    C = max(1, int(-(-loads.max() // P)))
    EROWS = -(-(NBLK * P * 3 * C) // (D + 1))
    TROWS = SROWS + EROWS

    perm = np.empty((NCORES, NPC), dtype=np.int64)
    perm[node_core, node_slot] = np.arange(N)

    # per-core upload rows: 0..NPC-1 node (h*norm) rows (permuted order,
    # int8 + pow2 per-row scale byte), then W as int16 split into a hi-byte
    # plane (scale byte in col D) and an offset-128 lo-byte plane.
    hn = h * norm
    vals = np.empty((NCORES, NPC, D), dtype=np.float32)
    for c in range(NCORES):
        vals[c] = hn[perm[c]]
    flat = vals.reshape(NCORES * NPC, D)
    s = np.abs(flat).max(axis=1, keepdims=True)
    s /= 127.0
    np.maximum(s, 1e-30, out=s)
    # pow2 scale, one exponent byte: s' = 2^((e-192)/16) >= s, e in [-128,127]
    e8 = np.clip(np.ceil(np.log2(s) * 16.0) + 192.0, -128, 127)
    sq = np.exp2((e8 - 192.0) / 16.0).astype(np.float32)
    np.multiply(flat, 1.0 / sq, out=flat)
    np.rint(flat, out=flat)
    np.clip(flat, -127, 127, out=flat)

    ws = np.abs(weight).max(axis=1, keepdims=True) / 16383.0
    np.maximum(ws, 1e-30, out=ws)
    we8 = np.clip(np.ceil(np.log2(ws) * 16.0) + 192.0, -128, 127)
    wsq = np.exp2((we8 - 192.0) / 16.0).astype(np.float32)
    q16 = np.clip(np.rint(weight / wsq), -16383, 16383).astype(np.int32)
    whi = (q16 >> 8).astype(np.int8)
    wlo = ((q16 & 0xFF) - 128).astype(np.int8)

    q = np.zeros((NCORES, SROWS, D + 1), dtype=np.int8)
    q[:, :NPC, :D] = flat.astype(np.int8).reshape(NCORES, NPC, D)
    q[:, :NPC, D] = e8.astype(np.int8).reshape(NCORES, NPC)
    q[:, NPC:NPC + WPC, :D] = whi.reshape(NCORES, WPC, D)
    q[:, NPC:NPC + WPC, D] = we8.astype(np.int8).reshape(NCORES, WPC)
    q[:, NPC + WPC:, :D] = wlo.reshape(NCORES, WPC, D)

    # node n lives at gathered row core*SROWS + slot
    srow = node_core[src] * SROWS + node_slot[src]
    gkey = node_core[dst] * NBLK + node_slot[dst] // P
    pslot = node_slot[dst] % P

    # edge bytes, offset by -128 into i8; idx lo/hi default to row 0,
    # slot defaults to 127 (padding sentinel, never matches a dst slot)
    edg_i8 = np.full((NCORES, NBLK, P, 3 * C), -128, dtype=np.int8)
    edg_i8[:, :, :, 2 * C:] = 127
    eorder = np.argsort(gkey, kind="stable")
    s_sorted = srow[eorder]
    p_sorted = pslot[eorder]
    g_sorted = gkey[eorder]
    starts = np.searchsorted(g_sorted, np.arange(NBUCK))
    rank = np.arange(len(g_sorted)) - starts[g_sorted]
    cc, bb, pp, kk = (g_sorted // NBLK, g_sorted % NBLK, rank % P, rank // P)
    edg_i8[cc, bb, pp, kk] = (s_sorted & 0xFF) - 128
    edg_i8[cc, bb, pp, C + kk] = (s_sorted >> 8) - 128
    edg_i8[cc, bb, pp, 2 * C + kk] = p_sorted - 128

    full = np.zeros((NCORES, TROWS, D + 1), dtype=np.int8)
    full[:, :SROWS] = q
    eb = edg_i8.reshape(NCORES, -1)
    full[:, SROWS:].reshape(NCORES, -1)[:, :eb.shape[1]] = eb

    in_maps = [{"hq": full[c]} for c in range(NCORES)]
    return C, in_maps, perm


def _unpack(res, norm, bias, perm):
    """Unpack planar 7-bit companded rows + u8 scale-exponent, applying
    dst-norm and bias: c = (q7-64)/63, v = sign(c)*|c|^(4/3),
    out[perm[c,slot]] = v * 2^((e-128)/16) * norm + bias."""
    norm = np.asarray(norm, dtype=np.float32).reshape(N, 1)
    bias = np.asarray(bias, dtype=np.float32)[None, :]
    G = D // 8
    out = np.empty((N, D), dtype=np.float32)
    for c in range(NCORES):
        raw = np.asarray(res[c]["out"])
        pl = raw[:, :PKB].astype(np.uint16).reshape(-1, 7, G)
        e = raw[:, PKB:PKB + 1].astype(np.float32)
        q = np.empty((raw.shape[0], 8, G), np.uint16)
        q[:, 0] = pl[:, 0] >> 1
        for j in range(1, 7):
            q[:, j] = ((pl[:, j - 1] << (7 - j)) | (pl[:, j] >> (j + 1))) & 0x7F
        q[:, 7] = pl[:, 6] & 0x7F
        vals = q.transpose(0, 2, 1).reshape(-1, D).astype(np.float32)
        cv = (vals - 64.0) / 63.0
        v = np.sign(cv) * np.abs(cv) ** (4.0 / 3.0)
        sc = np.exp2((e - 128.0) / 16.0)
        nodes = perm[c]
        out[nodes] = v * (sc * norm[nodes]) + bias
    return out


_NC_CACHE = {}


def kernel(h, norm, weight, bias, src, dst):
    h = np.asarray(h, dtype=np.float32)
    norm = np.asarray(norm, dtype=np.float32)
    weight = np.asarray(weight, dtype=np.float32)
    bias = np.asarray(bias, dtype=np.float32)
    C, in_maps, perm = _prep(h, norm, weight, bias, src, dst)
    nc = _NC_CACHE.get(C)
    if nc is None:
        nc = _NC_CACHE[C] = _build(C)
    res = run_bass_kernel_spmd(nc, in_maps, list(range(NCORES))).results
    return _unpack(res, norm, bias, perm)
